# revision 10
# baseline (speedup 1.0000x reference)
"""Trainium2 Bass kernel for nn_Backward_12094627905824 (MLP trunk + gumbel-argmax
mixture sampling). Data-parallel over 8 NeuronCores: batch B=262144 sharded
32768 rows/core; small MLP / head weights replicated.

Per batch row b:
  h = relu chain 3 -> 128 -> 256 -> 200
  mu/sig/pai[g,d] = heads (25 comps x 4 dims)
  idx[d] = argmax_g log(|pai|+eps) + gumbel[b,g,d]
  out[b,d] = rand[b,d]*|sig[idx,d]| + mu[idx,d]

Device reformulation (argmax-invariant): score t = pai_raw * exp(gumbel)
(exp precomputed on host), sc = |t|; one-hot oh = (sc >= groupmax); select
mu/sig via additive mask m = oh*1024 + val, reduce-max, subtract 1024.

Engine split per 512-row tile (balanced against the TimelineSim cost model):
  PE   : 15 matmuls (trunk + heads; head biases ride a ones-row in h3b)
  Act  : 5 relu+bias PSUM->SBUF copies
  DVE  : score mult (PSUM), group max, masked select stt + reduce-max
  Pool : |t| via abs_max, diff vs groupmax, one-hot u8, output combine
All head columns are (d*25+g)-ordered; eg is host-transposed to a per-partition
contiguous layout so DMAs move >=512B elements (no descriptor penalty) and are
batched 4-8 tiles per DMA instruction.
"""
import numpy as np

import concourse.bass as bass
import concourse.mybir as mybir
import bass_rust
from concourse.tile import TileContext
from concourse.bass_utils import run_bass_kernel_spmd

NCORES = 8
B, G, D = 262144, 25, 4
GD = G * D                       # 100
H1, H2, H3 = 128, 256, 200
BS = B // NCORES                 # 32768 rows per core
NB = 512                         # batch columns per compute tile
NT = BS // NB                    # 64 tiles
NSUB = NB // 128                 # 4 sub-blocks of 128 rows
BIG = 1024.0                     # additive mask offset (>> |mu|,|sig|)

F32 = mybir.dt.float32
F32R = mybir.dt.float32r
U8 = mybir.dt.uint8
A = mybir.AluOpType
AX = mybir.AxisListType


def _split_multi_waits(nc):
    # walrus CoreV3 codegen accepts only one sync-wait per instruction; Tile's
    # exit drain waits once per active proc. Split into single-wait drains.
    for bb in nc.main_func.blocks:
        insts = list(bb.instructions)
        out = []
        changed = False
        for inst in insts:
            si = inst.sync_info
            if si is not None and len(si.on_wait) > 1:
                waits = list(si.on_wait)
                for k, w in enumerate(waits[:-1]):
                    d = mybir.InstDrain(name=f"{inst.name}-sw{k}", ins=[], outs=[])
                    d.engine = inst.engine
                    d.sync_info = bass_rust.SyncInfo(on_wait=[w], on_update=[])
                    nc.register_instruction(d)
                    out.append(d)
                si.on_wait = [waits[-1]]
                changed = True
            out.append(inst)
        if changed:
            bb.instructions = out


def _build_nc():
    nc = bass.Bass(trn_type="TRN2")

    x0t = nc.dram_tensor("x0t", [3, BS], F32R, kind="ExternalInput")
    eg = nc.dram_tensor("eg", [128, NT * NSUB * GD], F32, kind="ExternalInput")
    rnd = nc.dram_tensor("rnd", [128, NT * NSUB * D], F32, kind="ExternalInput")
    w1t = nc.dram_tensor("w1t", [3, H1], F32R, kind="ExternalInput")
    b1 = nc.dram_tensor("b1", [H1, 1], F32, kind="ExternalInput")
    w2t = nc.dram_tensor("w2t", [H1, H2], F32R, kind="ExternalInput")
    b2 = nc.dram_tensor("b2", [H2, 1], F32, kind="ExternalInput")
    w3t = nc.dram_tensor("w3t", [H2, H3], F32R, kind="ExternalInput")
    b3 = nc.dram_tensor("b3", [H3, 1], F32, kind="ExternalInput")
    wha = nc.dram_tensor("wha", [128, 300], F32R, kind="ExternalInput")
    whb = nc.dram_tensor("whb", [73, 300], F32R, kind="ExternalInput")
    onesr = nc.dram_tensor("onesr", [1, NB], F32R, kind="ExternalInput")
    outd = nc.dram_tensor("outd", [128, NT * NSUB * D], F32, kind="ExternalOutput")

    from contextlib import ExitStack
    with TileContext(nc) as tc, ExitStack() as ctx:
        const = ctx.enter_context(tc.tile_pool(name="const", bufs=1))
        io = ctx.enter_context(tc.tile_pool(name="io", bufs=2))
        act = ctx.enter_context(tc.tile_pool(name="act", bufs=3))
        samp = ctx.enter_context(tc.tile_pool(name="samp", bufs=3))
        ptrunk = ctx.enter_context(tc.tile_pool(name="ptrunk", bufs=4, space="PSUM"))
        pheads = ctx.enter_context(tc.tile_pool(name="pheads", bufs=2, space="PSUM"))

        # --- load weights once ---
        w1t_s = const.tile([3, H1], F32R)
        nc.sync.dma_start(out=w1t_s, in_=w1t[:, :])
        b1_s = const.tile([H1, 1], F32)
        nc.sync.dma_start(out=b1_s, in_=b1[:, :])
        w2t_s = const.tile([H1, H2], F32R)
        nc.sync.dma_start(out=w2t_s, in_=w2t[:, :])
        b2a_s = const.tile([128, 1], F32, tag="b2a")
        nc.sync.dma_start(out=b2a_s, in_=b2[0:128, :])
        b2b_s = const.tile([128, 1], F32, tag="b2b")
        nc.sync.dma_start(out=b2b_s, in_=b2[128:256, :])
        w3ta_s = const.tile([128, H3], F32R, tag="w3ta")   # h2 feats 0:128
        nc.sync.dma_start(out=w3ta_s, in_=w3t[0:128, :])
        w3tb_s = const.tile([128, H3], F32R, tag="w3tb")   # h2 feats 128:256
        nc.sync.dma_start(out=w3tb_s, in_=w3t[128:256, :])
        b3a_s = const.tile([128, 1], F32, tag="b3a")
        nc.sync.dma_start(out=b3a_s, in_=b3[0:128, :])
        b3b_s = const.tile([72, 1], F32, tag="b3b")
        nc.sync.dma_start(out=b3b_s, in_=b3[128:200, :])
        wha_s = const.tile([128, 300], F32R, tag="wha")    # h3 feats 0:128
        nc.sync.dma_start(out=wha_s, in_=wha[:, :])
        whb_s = const.tile([73, 300], F32R, tag="whb")     # h3 feats 128:200 + bias row
        nc.sync.dma_start(out=whb_s, in_=whb[:, :])

        nbig_s = const.tile([128, 1], F32, tag="nbig")
        nc.gpsimd.memset(nbig_s, -BIG)

        # h3b tiles with a constant ones-row at partition 72 (manual 2-buffer)
        h3b_t = []
        for k in range(2):
            t = const.tile([73, NB], F32R, tag=f"h3b{k}")
            nc.sync.dma_start(out=t[72:73, :], in_=onesr[:, :])
            h3b_t.append(t)

        for it in range(NT):
            g4 = it % 4      # position in 4-tile eg/x DMA group
            g8 = it % 8      # position in 8-tile rnd/out DMA group

            # --- batched input DMAs ---
            if g4 == 0:
                x_s = io.tile([3, 4 * NB], F32R, tag="x")
                nc.sync.dma_start(out=x_s, in_=x0t[:, it * NB:(it + 4) * NB])
                eg_s = io.tile([128, 4, NSUB, GD], F32, tag="eg")
                nc.sync.dma_start(
                    out=eg_s,
                    in_=eg[:, it * NSUB * GD:(it + 4) * NSUB * GD]
                    .rearrange("p (t s e) -> p t s e", t=4, s=NSUB))
            if g8 == 0:
                rnd_s = io.tile([128, 8, NSUB, D], F32, tag="rnd")
                nc.sync.dma_start(
                    out=rnd_s,
                    in_=rnd[:, it * NSUB * D:(it + 8) * NSUB * D]
                    .rearrange("p (t s d) -> p t s d", t=8, s=NSUB))
                stage = io.tile([128, 8, NSUB, D], F32, tag="stage")

            # --- trunk matmuls + relus ---
            h1p = ptrunk.tile([128, NB], F32, tag="pt")
            nc.tensor.matmul(h1p, lhsT=w1t_s[:, :],
                             rhs=x_s[:, g4 * NB:(g4 + 1) * NB],
                             start=True, stop=True)
            h1 = act.tile([128, NB], F32R, tag="h1")
            nc.scalar.activation(h1, h1p, func=mybir.ActivationFunctionType.Relu,
                                 bias=b1_s[:, :], scale=1.0)

            h2ap = ptrunk.tile([128, NB], F32, tag="pt")
            nc.tensor.matmul(h2ap, lhsT=w2t_s[:, 0:128], rhs=h1[:, :],
                             start=True, stop=True)
            h2a = act.tile([128, NB], F32R, tag="h2a")
            nc.scalar.activation(h2a, h2ap, func=mybir.ActivationFunctionType.Relu,
                                 bias=b2a_s[:, :], scale=1.0)

            h2bp = ptrunk.tile([128, NB], F32, tag="pt")
            nc.tensor.matmul(h2bp, lhsT=w2t_s[:, 128:256], rhs=h1[:, :],
                             start=True, stop=True)
            h2b = act.tile([128, NB], F32R, tag="h2b")
            nc.scalar.activation(h2b, h2bp, func=mybir.ActivationFunctionType.Relu,
                                 bias=b2b_s[:, :], scale=1.0)

            h3ap = ptrunk.tile([128, NB], F32, tag="pt")
            nc.tensor.matmul(h3ap, lhsT=w3ta_s[:, 0:128], rhs=h2a[:, :],
                             start=True, stop=False)
            nc.tensor.matmul(h3ap, lhsT=w3tb_s[:, 0:128], rhs=h2b[:, :],
                             start=False, stop=True)
            h3a = act.tile([128, NB], F32R, tag="h3a")
            nc.scalar.activation(h3a, h3ap, func=mybir.ActivationFunctionType.Relu,
                                 bias=b3a_s[:, :], scale=1.0)

            h3bp = ptrunk.tile([72, NB], F32, tag="pt")
            nc.tensor.matmul(h3bp, lhsT=w3ta_s[:, 128:200], rhs=h2a[:, :],
                             start=True, stop=False)
            nc.tensor.matmul(h3bp, lhsT=w3tb_s[:, 128:200], rhs=h2b[:, :],
                             start=False, stop=True)
            h3b = h3b_t[it % 2]
            nc.scalar.activation(h3b[0:72, :], h3bp,
                                 func=mybir.ActivationFunctionType.Relu,
                                 bias=b3b_s[:, :], scale=1.0)

            # --- heads: two psum tiles of [128, 2sub, 512]; cols 0:300 =
            #     [mu|sig|pai] in (d*25+g) order; bias rides the ones-row ---
            ph = []
            for half in range(2):
                pht = pheads.tile([128, 2, 512], F32, tag="ph")
                for j in range(2):
                    s = half * 2 + j
                    c0, c1 = s * 128, (s + 1) * 128
                    nc.tensor.matmul(pht[:, j, 0:300], lhsT=h3a[:, c0:c1],
                                     rhs=wha_s[:, :], start=True, stop=False)
                    nc.tensor.matmul(pht[:, j, 0:300], lhsT=h3b[:, c0:c1],
                                     rhs=whb_s[:, :], start=False, stop=True)
                ph.append(pht)

            egs = eg_s[:, g4]                       # [128, NSUB, GD]
            # --- score t = pai_raw * eg  (DVE; one PSUM input each) ---
            t = samp.tile([128, NSUB, GD], F32, tag="t")
            nc.vector.tensor_tensor(out=t[:, 0:2], in0=ph[0][:, :, 200:300],
                                    in1=egs[:, 0:2], op=A.mult)
            nc.vector.tensor_tensor(out=t[:, 2:4], in0=ph[1][:, :, 200:300],
                                    in1=egs[:, 2:4], op=A.mult)

            # --- sc = t*t (Pool; squares rank like |t| since eg > 0) ---
            sc = samp.tile([128, NSUB, GD], F32, tag="sc")
            nc.gpsimd.tensor_tensor(out=sc, in0=t, in1=t, op=A.mult)

            # --- group max over g (DVE reduce on (d g) layout) ---
            smax = samp.tile([128, NSUB, D], F32, tag="smax")
            sc_v = sc.rearrange("p s (d g) -> p s d g", d=D)
            nc.vector.tensor_reduce(smax, sc_v, axis=AX.X, op=A.max)

            # --- diff = sc - smax (Pool, per-sub to keep APs <=3D) ---
            dif = samp.tile([128, NSUB, GD], F32, tag="dif")
            for s in range(NSUB):
                smax_b = smax[:, s].unsqueeze(2).broadcast_to([128, D, G])
                nc.gpsimd.tensor_tensor(
                    out=dif[:, s].rearrange("p (d g) -> p d g", d=D),
                    in0=sc_v[:, s], in1=smax_b, op=A.subtract)
            oh = samp.tile([128, NSUB, GD], U8, tag="oh")
            nc.gpsimd.tensor_scalar(out=oh, in0=dif, scalar1=0.0, scalar2=None,
                                    op0=A.is_ge)

            # --- masked select: m = oh*BIG + [mu|sig]; reduce-max (DVE).
            #     m is h-major [128, 2, NSUB, GD]; one stt per (half, head) ---
            m = samp.tile([128, 2, NSUB, GD], F32, tag="m")
            for half in range(2):
                s0 = 2 * half
                for h in range(2):
                    nc.vector.scalar_tensor_tensor(
                        out=m[:, h, s0:s0 + 2],
                        in0=oh[:, s0:s0 + 2], scalar=BIG, op0=A.mult, op1=A.add,
                        in1=ph[half][:, :, h * GD:(h + 1) * GD])
            sel = samp.tile([128, 2, NSUB, D], F32, tag="sel")
            nc.vector.tensor_reduce(
                sel, m.rearrange("p h s (d g) -> p h s d g", d=D),
                axis=AX.X, op=A.max)

            # --- out = rnd*|sig_sel| + mu_sel  (Pool) ---
            sg = samp.tile([128, NSUB, D], F32, tag="sg")
            nc.scalar.activation(sg, sel[:, 1], func=mybir.ActivationFunctionType.Abs,
                                 bias=nbig_s[:, :], scale=1.0)
            t1 = samp.tile([128, NSUB, D], F32, tag="t1")
            nc.gpsimd.tensor_tensor(out=t1, in0=sg, in1=rnd_s[:, g8], op=A.mult)
            t2 = samp.tile([128, NSUB, D], F32, tag="t2")
            nc.gpsimd.tensor_tensor(out=t2, in0=t1, in1=sel[:, 0], op=A.add)
            nc.gpsimd.tensor_scalar(out=stage[:, g8], in0=t2, scalar1=-BIG,
                                    scalar2=None, op0=A.add)

            if g8 == 7:
                nc.sync.dma_start(
                    out=outd[:, (it - 7) * NSUB * D:(it + 1) * NSUB * D]
                    .rearrange("p (t s d) -> p t s d", t=8, s=NSUB),
                    in_=stage)

    _split_multi_waits(nc)
    return nc


_NC_CACHE = None
LAST_RESULT = None


def kernel(x0, rand, gumbel, W1, b1, W2, b2, W3, b3,
           Wmu, bmu, Wsig, bsig, Wpai, bpai):
    global _NC_CACHE, LAST_RESULT
    if _NC_CACHE is None:
        _NC_CACHE = _build_nc()
    nc = _NC_CACHE

    x0 = np.ascontiguousarray(np.asarray(x0, np.float32))
    rand = np.ascontiguousarray(np.asarray(rand, np.float32))
    gumbel = np.asarray(gumbel, np.float32)

    # Head weight block [201, 300]: rows 0..199 = h3 feats, row 200 = bias.
    # col = head*100 + d*25 + g  (d-major, g contiguous for segmented ops)
    WH = np.zeros((H3 + 1, 300), np.float32)
    for hd, (W, bvec) in enumerate([(Wmu, bmu), (Wsig, bsig), (Wpai, bpai)]):
        Wt = np.asarray(W, np.float32).transpose(1, 0, 2).reshape(GD, H3)  # (d g) rows
        WH[:H3, hd * GD:(hd + 1) * GD] = Wt.T
        WH[H3, hd * GD:(hd + 1) * GD] = np.asarray(bvec, np.float32).T.reshape(GD)

    wmats = {
        "w1t": np.ascontiguousarray(np.asarray(W1, np.float32).T),
        "b1": np.asarray(b1, np.float32).reshape(H1, 1),
        "w2t": np.ascontiguousarray(np.asarray(W2, np.float32).T),
        "b2": np.asarray(b2, np.float32).reshape(H2, 1),
        "w3t": np.ascontiguousarray(np.asarray(W3, np.float32).T),
        "b3": np.asarray(b3, np.float32).reshape(H3, 1),
        "wha": np.ascontiguousarray(WH[0:128]),
        "onesr": np.ones((1, NB), np.float32),
        "whb": np.ascontiguousarray(WH[128:201]),
    }

    eg_full = np.exp(gumbel, dtype=np.float32)      # [B, G, D]

    in_maps = []
    for c in range(NCORES):
        sl = slice(c * BS, (c + 1) * BS)
        # eg: [BS,G,D] -> (d g) cols -> [p, it, s, e] contiguous per partition
        egc = eg_full[sl].transpose(0, 2, 1).reshape(BS, GD)
        egc = egc.reshape(NT, NSUB, 128, GD).transpose(2, 0, 1, 3)
        rndc = rand[sl].reshape(NT, NSUB, 128, D).transpose(2, 0, 1, 3)
        m = {
            "x0t": np.ascontiguousarray(x0[sl].T),
            "eg": np.ascontiguousarray(egc.reshape(128, NT * NSUB * GD)),
            "rnd": np.ascontiguousarray(rndc.reshape(128, NT * NSUB * D)),
        }
        m.update(wmats)
        in_maps.append(m)

    res = run_bass_kernel_spmd(nc, in_maps, core_ids=list(range(NCORES)))
    LAST_RESULT = res
    outs = []
    for c in range(NCORES):
        oc = res.results[c]["outd"].reshape(128, NT, NSUB, D)
        outs.append(oc.transpose(1, 2, 0, 3).reshape(BS, D))
    return np.concatenate(outs, axis=0).astype(np.float32)


# revision 11
# speedup vs baseline: 1.0692x; 1.0692x over previous
"""Trainium2 Bass kernel for nn_Backward_12094627905824 (MLP trunk + gumbel-argmax
mixture sampling). Data-parallel over 8 NeuronCores: batch B=262144 sharded
32768 rows/core; small MLP / head weights replicated.

Per batch row b:
  h = relu chain 3 -> 128 -> 256 -> 200
  mu/sig/pai[g,d] = heads (25 comps x 4 dims)
  idx[d] = argmax_g log(|pai|+eps) + gumbel[b,g,d]
  out[b,d] = rand[b,d]*|sig[idx,d]| + mu[idx,d]

Device reformulation (argmax-invariant): score t = pai_raw * exp(gumbel)
(exp precomputed on host), sc = |t|; one-hot oh = (sc >= groupmax); select
mu/sig via additive mask m = oh*1024 + val, reduce-max, subtract 1024.

Engine split per 512-row tile (balanced against the TimelineSim cost model):
  PE   : 15 matmuls (trunk + heads; head biases ride a ones-row in h3b)
  Act  : 5 relu+bias PSUM->SBUF copies
  DVE  : score mult (PSUM), group max, masked select stt + reduce-max
  Pool : |t| via abs_max, diff vs groupmax, one-hot u8, output combine
All head columns are (d*25+g)-ordered; eg is host-transposed to a per-partition
contiguous layout so DMAs move >=512B elements (no descriptor penalty) and are
batched 4-8 tiles per DMA instruction.
"""
import numpy as np

import concourse.bass as bass
import concourse.mybir as mybir
import bass_rust
from concourse.tile import TileContext
from concourse.bass_utils import run_bass_kernel_spmd

NCORES = 8
B, G, D = 262144, 25, 4
GD = G * D                       # 100
H1, H2, H3 = 128, 256, 200
BS = B // NCORES                 # 32768 rows per core
NB = 512                         # batch columns per compute tile
NT = BS // NB                    # 64 tiles
NSUB = NB // 128                 # 4 sub-blocks of 128 rows
BIG = 1024.0                     # additive mask offset (>> |mu|,|sig|)

F32 = mybir.dt.float32
F32R = mybir.dt.float32r
U8 = mybir.dt.uint8
A = mybir.AluOpType
AX = mybir.AxisListType


def _split_multi_waits(nc):
    # walrus CoreV3 codegen accepts only one sync-wait per instruction; Tile's
    # exit drain waits once per active proc. Split into single-wait drains.
    for bb in nc.main_func.blocks:
        insts = list(bb.instructions)
        out = []
        changed = False
        for inst in insts:
            si = inst.sync_info
            if si is not None and len(si.on_wait) > 1:
                waits = list(si.on_wait)
                for k, w in enumerate(waits[:-1]):
                    d = mybir.InstDrain(name=f"{inst.name}-sw{k}", ins=[], outs=[])
                    d.engine = inst.engine
                    d.sync_info = bass_rust.SyncInfo(on_wait=[w], on_update=[])
                    nc.register_instruction(d)
                    out.append(d)
                si.on_wait = [waits[-1]]
                changed = True
            out.append(inst)
        if changed:
            bb.instructions = out


def _build_nc():
    nc = bass.Bass(trn_type="TRN2")

    x0t = nc.dram_tensor("x0t", [3, BS], F32R, kind="ExternalInput")
    eg = nc.dram_tensor("eg", [128, NT * NSUB * GD], F32, kind="ExternalInput")
    rnd = nc.dram_tensor("rnd", [128, NT * NSUB * D], F32, kind="ExternalInput")
    w1t = nc.dram_tensor("w1t", [3, H1], F32R, kind="ExternalInput")
    b1 = nc.dram_tensor("b1", [H1, 1], F32, kind="ExternalInput")
    w2t = nc.dram_tensor("w2t", [H1, H2], F32R, kind="ExternalInput")
    b2 = nc.dram_tensor("b2", [H2, 1], F32, kind="ExternalInput")
    w3t = nc.dram_tensor("w3t", [H2, H3], F32R, kind="ExternalInput")
    b3 = nc.dram_tensor("b3", [H3, 1], F32, kind="ExternalInput")
    wha = nc.dram_tensor("wha", [128, 300], F32R, kind="ExternalInput")
    whb = nc.dram_tensor("whb", [73, 300], F32R, kind="ExternalInput")
    onesr = nc.dram_tensor("onesr", [1, NB], F32R, kind="ExternalInput")
    outd = nc.dram_tensor("outd", [128, NT * NSUB * D], F32, kind="ExternalOutput")

    from contextlib import ExitStack
    with TileContext(nc) as tc, ExitStack() as ctx:
        const = ctx.enter_context(tc.tile_pool(name="const", bufs=1))
        io = ctx.enter_context(tc.tile_pool(name="io", bufs=3))
        act = ctx.enter_context(tc.tile_pool(name="act", bufs=4))
        samp = ctx.enter_context(tc.tile_pool(name="samp", bufs=4))
        ptrunk = ctx.enter_context(tc.tile_pool(name="ptrunk", bufs=2, space="PSUM"))
        pheads = ctx.enter_context(tc.tile_pool(name="pheads", bufs=3, space="PSUM"))

        # --- load weights once ---
        w1t_s = const.tile([3, H1], F32R)
        nc.sync.dma_start(out=w1t_s, in_=w1t[:, :])
        b1_s = const.tile([H1, 1], F32)
        nc.sync.dma_start(out=b1_s, in_=b1[:, :])
        w2t_s = const.tile([H1, H2], F32R)
        nc.sync.dma_start(out=w2t_s, in_=w2t[:, :])
        b2a_s = const.tile([128, 1], F32, tag="b2a")
        nc.sync.dma_start(out=b2a_s, in_=b2[0:128, :])
        b2b_s = const.tile([128, 1], F32, tag="b2b")
        nc.sync.dma_start(out=b2b_s, in_=b2[128:256, :])
        w3ta_s = const.tile([128, H3], F32R, tag="w3ta")   # h2 feats 0:128
        nc.sync.dma_start(out=w3ta_s, in_=w3t[0:128, :])
        w3tb_s = const.tile([128, H3], F32R, tag="w3tb")   # h2 feats 128:256
        nc.sync.dma_start(out=w3tb_s, in_=w3t[128:256, :])
        b3a_s = const.tile([128, 1], F32, tag="b3a")
        nc.sync.dma_start(out=b3a_s, in_=b3[0:128, :])
        b3b_s = const.tile([72, 1], F32, tag="b3b")
        nc.sync.dma_start(out=b3b_s, in_=b3[128:200, :])
        wha_s = const.tile([128, 300], F32R, tag="wha")    # h3 feats 0:128
        nc.sync.dma_start(out=wha_s, in_=wha[:, :])
        whb_s = const.tile([73, 300], F32R, tag="whb")     # h3 feats 128:200 + bias row
        nc.sync.dma_start(out=whb_s, in_=whb[:, :])

        nbig_s = const.tile([128, 1], F32, tag="nbig")
        nc.gpsimd.memset(nbig_s, -BIG)

        # h3b tiles with a constant ones-row at partition 72 (manual 2-buffer)
        h3b_t = []
        for k in range(3):
            t = const.tile([73, NB], F32R, tag=f"h3b{k}")
            nc.sync.dma_start(out=t[72:73, :], in_=onesr[:, :])
            h3b_t.append(t)

        for it in range(NT):
            g4 = it % 4      # position in 4-tile eg/x DMA group
            g8 = it % 8      # position in 8-tile rnd/out DMA group

            # --- batched input DMAs ---
            if g4 == 0:
                x_s = io.tile([3, 4 * NB], F32R, tag="x")
                nc.sync.dma_start(out=x_s, in_=x0t[:, it * NB:(it + 4) * NB])
                eg_s = io.tile([128, 4, NSUB, GD], F32, tag="eg")
                nc.sync.dma_start(
                    out=eg_s,
                    in_=eg[:, it * NSUB * GD:(it + 4) * NSUB * GD]
                    .rearrange("p (t s e) -> p t s e", t=4, s=NSUB))
            if g8 == 0:
                rnd_s = io.tile([128, 8, NSUB, D], F32, tag="rnd")
                nc.sync.dma_start(
                    out=rnd_s,
                    in_=rnd[:, it * NSUB * D:(it + 8) * NSUB * D]
                    .rearrange("p (t s d) -> p t s d", t=8, s=NSUB))
                stage = io.tile([128, 8, NSUB, D], F32, tag="stage")

            # --- trunk matmuls + relus ---
            h1p = ptrunk.tile([128, NB], F32, tag="pt")
            nc.tensor.matmul(h1p, lhsT=w1t_s[:, :],
                             rhs=x_s[:, g4 * NB:(g4 + 1) * NB],
                             start=True, stop=True)
            h1 = act.tile([128, NB], F32R, tag="h1")
            nc.scalar.activation(h1, h1p, func=mybir.ActivationFunctionType.Relu,
                                 bias=b1_s[:, :], scale=1.0)

            h2ap = ptrunk.tile([128, NB], F32, tag="pt")
            nc.tensor.matmul(h2ap, lhsT=w2t_s[:, 0:128], rhs=h1[:, :],
                             start=True, stop=True)
            h2a = act.tile([128, NB], F32R, tag="h2a")
            nc.scalar.activation(h2a, h2ap, func=mybir.ActivationFunctionType.Relu,
                                 bias=b2a_s[:, :], scale=1.0)

            h2bp = ptrunk.tile([128, NB], F32, tag="pt")
            nc.tensor.matmul(h2bp, lhsT=w2t_s[:, 128:256], rhs=h1[:, :],
                             start=True, stop=True)
            h2b = act.tile([128, NB], F32R, tag="h2b")
            nc.scalar.activation(h2b, h2bp, func=mybir.ActivationFunctionType.Relu,
                                 bias=b2b_s[:, :], scale=1.0)

            h3ap = ptrunk.tile([128, NB], F32, tag="pt")
            nc.tensor.matmul(h3ap, lhsT=w3ta_s[:, 0:128], rhs=h2a[:, :],
                             start=True, stop=False)
            nc.tensor.matmul(h3ap, lhsT=w3tb_s[:, 0:128], rhs=h2b[:, :],
                             start=False, stop=True)
            h3a = act.tile([128, NB], F32R, tag="h3a")
            nc.scalar.activation(h3a, h3ap, func=mybir.ActivationFunctionType.Relu,
                                 bias=b3a_s[:, :], scale=1.0)

            h3bp = ptrunk.tile([72, NB], F32, tag="pt")
            nc.tensor.matmul(h3bp, lhsT=w3ta_s[:, 128:200], rhs=h2a[:, :],
                             start=True, stop=False)
            nc.tensor.matmul(h3bp, lhsT=w3tb_s[:, 128:200], rhs=h2b[:, :],
                             start=False, stop=True)
            h3b = h3b_t[it % 3]
            nc.scalar.activation(h3b[0:72, :], h3bp,
                                 func=mybir.ActivationFunctionType.Relu,
                                 bias=b3b_s[:, :], scale=1.0)

            # --- heads: two psum tiles of [128, 2sub, 512]; cols 0:300 =
            #     [mu|sig|pai] in (d*25+g) order; bias rides the ones-row ---
            ph = []
            for half in range(2):
                pht = pheads.tile([128, 2, 512], F32, tag="ph")
                for j in range(2):
                    s = half * 2 + j
                    c0, c1 = s * 128, (s + 1) * 128
                    nc.tensor.matmul(pht[:, j, 0:300], lhsT=h3a[:, c0:c1],
                                     rhs=wha_s[:, :], start=True, stop=False)
                    nc.tensor.matmul(pht[:, j, 0:300], lhsT=h3b[:, c0:c1],
                                     rhs=whb_s[:, :], start=False, stop=True)
                ph.append(pht)

            egs = eg_s[:, g4]                       # [128, NSUB, GD]
            # --- score t = pai_raw * eg  (DVE; one PSUM input each) ---
            t = samp.tile([128, NSUB, GD], F32, tag="t")
            nc.vector.tensor_tensor(out=t[:, 0:2], in0=ph[0][:, :, 200:300],
                                    in1=egs[:, 0:2], op=A.mult)
            nc.vector.tensor_tensor(out=t[:, 2:4], in0=ph[1][:, :, 200:300],
                                    in1=egs[:, 2:4], op=A.mult)

            # --- sc = t*t (Pool; squares rank like |t| since eg > 0) ---
            sc = samp.tile([128, NSUB, GD], F32, tag="sc")
            nc.gpsimd.tensor_tensor(out=sc, in0=t, in1=t, op=A.mult)

            # --- group max over g (DVE reduce on (d g) layout) ---
            smax = samp.tile([128, NSUB, D], F32, tag="smax")
            sc_v = sc.rearrange("p s (d g) -> p s d g", d=D)
            nc.vector.tensor_reduce(smax, sc_v, axis=AX.X, op=A.max)

            # --- diff = sc - smax (Pool, per-sub to keep APs <=3D) ---
            dif = samp.tile([128, NSUB, GD], F32, tag="dif")
            for s in range(NSUB):
                smax_b = smax[:, s].unsqueeze(2).broadcast_to([128, D, G])
                nc.gpsimd.tensor_tensor(
                    out=dif[:, s].rearrange("p (d g) -> p d g", d=D),
                    in0=sc_v[:, s], in1=smax_b, op=A.subtract)
            oh = samp.tile([128, NSUB, GD], U8, tag="oh")
            nc.gpsimd.tensor_scalar(out=oh, in0=dif, scalar1=0.0, scalar2=None,
                                    op0=A.is_ge)

            # --- masked select: m = oh*BIG + [mu|sig]; reduce-max (DVE).
            #     m is h-major [128, 2, NSUB, GD]; one stt per (half, head) ---
            m = samp.tile([128, 2, NSUB, GD], F32, tag="m")
            for half in range(2):
                s0 = 2 * half
                for h in range(2):
                    nc.vector.scalar_tensor_tensor(
                        out=m[:, h, s0:s0 + 2],
                        in0=oh[:, s0:s0 + 2], scalar=BIG, op0=A.mult, op1=A.add,
                        in1=ph[half][:, :, h * GD:(h + 1) * GD])
            sel = samp.tile([128, 2, NSUB, D], F32, tag="sel")
            nc.vector.tensor_reduce(
                sel, m.rearrange("p h s (d g) -> p h s d g", d=D),
                axis=AX.X, op=A.max)

            # --- out = rnd*|sig_sel| + mu_sel  (Pool) ---
            sg = samp.tile([128, NSUB, D], F32, tag="sg")
            nc.scalar.activation(sg, sel[:, 1], func=mybir.ActivationFunctionType.Abs,
                                 bias=nbig_s[:, :], scale=1.0)
            t1 = samp.tile([128, NSUB, D], F32, tag="t1")
            nc.gpsimd.tensor_tensor(out=t1, in0=sg, in1=rnd_s[:, g8], op=A.mult)
            t2 = samp.tile([128, NSUB, D], F32, tag="t2")
            nc.gpsimd.tensor_tensor(out=t2, in0=t1, in1=sel[:, 0], op=A.add)
            nc.gpsimd.tensor_scalar(out=stage[:, g8], in0=t2, scalar1=-BIG,
                                    scalar2=None, op0=A.add)

            if g8 == 7:
                nc.sync.dma_start(
                    out=outd[:, (it - 7) * NSUB * D:(it + 1) * NSUB * D]
                    .rearrange("p (t s d) -> p t s d", t=8, s=NSUB),
                    in_=stage)

    _split_multi_waits(nc)
    return nc


_NC_CACHE = None
LAST_RESULT = None


def kernel(x0, rand, gumbel, W1, b1, W2, b2, W3, b3,
           Wmu, bmu, Wsig, bsig, Wpai, bpai):
    global _NC_CACHE, LAST_RESULT
    if _NC_CACHE is None:
        _NC_CACHE = _build_nc()
    nc = _NC_CACHE

    x0 = np.ascontiguousarray(np.asarray(x0, np.float32))
    rand = np.ascontiguousarray(np.asarray(rand, np.float32))
    gumbel = np.asarray(gumbel, np.float32)

    # Head weight block [201, 300]: rows 0..199 = h3 feats, row 200 = bias.
    # col = head*100 + d*25 + g  (d-major, g contiguous for segmented ops)
    WH = np.zeros((H3 + 1, 300), np.float32)
    for hd, (W, bvec) in enumerate([(Wmu, bmu), (Wsig, bsig), (Wpai, bpai)]):
        Wt = np.asarray(W, np.float32).transpose(1, 0, 2).reshape(GD, H3)  # (d g) rows
        WH[:H3, hd * GD:(hd + 1) * GD] = Wt.T
        WH[H3, hd * GD:(hd + 1) * GD] = np.asarray(bvec, np.float32).T.reshape(GD)

    wmats = {
        "w1t": np.ascontiguousarray(np.asarray(W1, np.float32).T),
        "b1": np.asarray(b1, np.float32).reshape(H1, 1),
        "w2t": np.ascontiguousarray(np.asarray(W2, np.float32).T),
        "b2": np.asarray(b2, np.float32).reshape(H2, 1),
        "w3t": np.ascontiguousarray(np.asarray(W3, np.float32).T),
        "b3": np.asarray(b3, np.float32).reshape(H3, 1),
        "wha": np.ascontiguousarray(WH[0:128]),
        "onesr": np.ones((1, NB), np.float32),
        "whb": np.ascontiguousarray(WH[128:201]),
    }

    eg_full = np.exp(gumbel, dtype=np.float32)      # [B, G, D]

    in_maps = []
    for c in range(NCORES):
        sl = slice(c * BS, (c + 1) * BS)
        # eg: [BS,G,D] -> (d g) cols -> [p, it, s, e] contiguous per partition
        egc = eg_full[sl].transpose(0, 2, 1).reshape(BS, GD)
        egc = egc.reshape(NT, NSUB, 128, GD).transpose(2, 0, 1, 3)
        rndc = rand[sl].reshape(NT, NSUB, 128, D).transpose(2, 0, 1, 3)
        m = {
            "x0t": np.ascontiguousarray(x0[sl].T),
            "eg": np.ascontiguousarray(egc.reshape(128, NT * NSUB * GD)),
            "rnd": np.ascontiguousarray(rndc.reshape(128, NT * NSUB * D)),
        }
        m.update(wmats)
        in_maps.append(m)

    res = run_bass_kernel_spmd(nc, in_maps, core_ids=list(range(NCORES)))
    LAST_RESULT = res
    outs = []
    for c in range(NCORES):
        oc = res.results[c]["outd"].reshape(128, NT, NSUB, D)
        outs.append(oc.transpose(1, 2, 0, 3).reshape(BS, D))
    return np.concatenate(outs, axis=0).astype(np.float32)


# revision 14
# speedup vs baseline: 1.2727x; 1.1904x over previous
"""Trainium2 Bass kernel for nn_Backward_12094627905824 (MLP trunk + gumbel-argmax
mixture sampling). Data-parallel over 8 NeuronCores: batch B=262144 sharded
32768 rows/core; small MLP / head weights replicated.

Per batch row b:
  h = relu chain 3 -> 128 -> 256 -> 200
  mu/sig/pai[g,d] = heads (25 comps x 4 dims)
  idx[d] = argmax_g log(|pai|+eps) + gumbel[b,g,d]
  out[b,d] = rand[b,d]*|sig[idx,d]| + mu[idx,d]

Device reformulation (argmax-invariant): score t = pai_raw * exp(gumbel)
(exp precomputed on host), sc = |t|; one-hot oh = (sc >= groupmax); select
mu/sig via additive mask m = oh*1024 + val, reduce-max, subtract 1024.

Engine split per 512-row tile (balanced against the TimelineSim cost model):
  PE   : 15 matmuls (trunk + heads; head biases ride a ones-row in h3b)
  Act  : 5 relu+bias PSUM->SBUF copies
  DVE  : score mult (PSUM), group max, masked select stt + reduce-max
  Pool : |t| via abs_max, diff vs groupmax, one-hot u8, output combine
All head columns are (d*25+g)-ordered; eg is host-transposed to a per-partition
contiguous layout so DMAs move >=512B elements (no descriptor penalty) and are
batched 4-8 tiles per DMA instruction.
"""
import numpy as np

import concourse.bass as bass
import concourse.mybir as mybir
import bass_rust
from concourse.tile import TileContext
from concourse.bass_utils import run_bass_kernel_spmd

NCORES = 8
B, G, D = 262144, 25, 4
GD = G * D                       # 100
H1, H2, H3 = 128, 256, 200
BS = B // NCORES                 # 32768 rows per core
NB = 512                         # batch columns per compute tile
NT = BS // NB                    # 64 tiles
NSUB = NB // 128                 # 4 sub-blocks of 128 rows
BIG = 1024.0                     # additive mask offset (>> |mu|,|sig|)

F32 = mybir.dt.float32
F32R = mybir.dt.float32r
U8 = mybir.dt.uint8
F16 = mybir.dt.float16
A = mybir.AluOpType
AX = mybir.AxisListType


def _split_multi_waits(nc):
    # walrus CoreV3 codegen accepts only one sync-wait per instruction; Tile's
    # exit drain waits once per active proc. Split into single-wait drains.
    for bb in nc.main_func.blocks:
        insts = list(bb.instructions)
        out = []
        changed = False
        for inst in insts:
            si = inst.sync_info
            if si is not None and len(si.on_wait) > 1:
                waits = list(si.on_wait)
                for k, w in enumerate(waits[:-1]):
                    d = mybir.InstDrain(name=f"{inst.name}-sw{k}", ins=[], outs=[])
                    d.engine = inst.engine
                    d.sync_info = bass_rust.SyncInfo(on_wait=[w], on_update=[])
                    nc.register_instruction(d)
                    out.append(d)
                si.on_wait = [waits[-1]]
                changed = True
            out.append(inst)
        if changed:
            bb.instructions = out


def _build_nc():
    nc = bass.Bass(trn_type="TRN2")

    x0t = nc.dram_tensor("x0t", [3, BS], F32R, kind="ExternalInput")
    eg = nc.dram_tensor("eg", [128, NT * NSUB * GD], F32, kind="ExternalInput")
    rnd = nc.dram_tensor("rnd", [128, NT * NSUB * D], F32, kind="ExternalInput")
    w1t = nc.dram_tensor("w1t", [3, H1], F32R, kind="ExternalInput")
    b1 = nc.dram_tensor("b1", [H1, 1], F32, kind="ExternalInput")
    w2t = nc.dram_tensor("w2t", [H1, H2], F32R, kind="ExternalInput")
    b2 = nc.dram_tensor("b2", [H2, 1], F32, kind="ExternalInput")
    w3t = nc.dram_tensor("w3t", [H2, H3], F32R, kind="ExternalInput")
    b3 = nc.dram_tensor("b3", [H3, 1], F32, kind="ExternalInput")
    wha = nc.dram_tensor("wha", [128, 300], F32R, kind="ExternalInput")
    whb = nc.dram_tensor("whb", [73, 300], F32R, kind="ExternalInput")
    onesr = nc.dram_tensor("onesr", [1, NB], F32R, kind="ExternalInput")
    outd = nc.dram_tensor("outd", [128, NT * NSUB * D], F32, kind="ExternalOutput")

    from contextlib import ExitStack
    with TileContext(nc) as tc, ExitStack() as ctx:
        const = ctx.enter_context(tc.tile_pool(name="const", bufs=1))
        io = ctx.enter_context(tc.tile_pool(name="io", bufs=3))
        act = ctx.enter_context(tc.tile_pool(name="act", bufs=4))
        samp = ctx.enter_context(tc.tile_pool(name="samp", bufs=4))
        ptrunk = ctx.enter_context(tc.tile_pool(name="ptrunk", bufs=2, space="PSUM"))
        pheads = ctx.enter_context(tc.tile_pool(name="pheads", bufs=3, space="PSUM"))

        # --- load weights once ---
        w1t_s = const.tile([3, H1], F32R)
        nc.sync.dma_start(out=w1t_s, in_=w1t[:, :])
        b1_s = const.tile([H1, 1], F32)
        nc.sync.dma_start(out=b1_s, in_=b1[:, :])
        w2t_s = const.tile([H1, H2], F32R)
        nc.sync.dma_start(out=w2t_s, in_=w2t[:, :])
        b2a_s = const.tile([128, 1], F32, tag="b2a")
        nc.sync.dma_start(out=b2a_s, in_=b2[0:128, :])
        b2b_s = const.tile([128, 1], F32, tag="b2b")
        nc.sync.dma_start(out=b2b_s, in_=b2[128:256, :])
        w3ta_s = const.tile([128, H3], F32R, tag="w3ta")   # h2 feats 0:128
        nc.sync.dma_start(out=w3ta_s, in_=w3t[0:128, :])
        w3tb_s = const.tile([128, H3], F32R, tag="w3tb")   # h2 feats 128:256
        nc.sync.dma_start(out=w3tb_s, in_=w3t[128:256, :])
        b3a_s = const.tile([128, 1], F32, tag="b3a")
        nc.sync.dma_start(out=b3a_s, in_=b3[0:128, :])
        b3b_s = const.tile([72, 1], F32, tag="b3b")
        nc.sync.dma_start(out=b3b_s, in_=b3[128:200, :])
        wha_s = const.tile([128, 300], F32R, tag="wha")    # h3 feats 0:128
        nc.sync.dma_start(out=wha_s, in_=wha[:, :])
        whb_s = const.tile([73, 300], F32R, tag="whb")     # h3 feats 128:200 + bias row
        nc.sync.dma_start(out=whb_s, in_=whb[:, :])

        # h3b tiles with a constant ones-row at partition 72 (manual 2-buffer)
        h3b_t = []
        for k in range(3):
            t = const.tile([73, NB], F32R, tag=f"h3b{k}")
            nc.sync.dma_start(out=t[72:73, :], in_=onesr[:, :])
            h3b_t.append(t)

        for it in range(NT):
            g4 = it % 4      # position in 4-tile eg/x DMA group
            g8 = it % 8      # position in 8-tile rnd/out DMA group

            # --- batched input DMAs ---
            if g4 == 0:
                x_s = io.tile([3, 4 * NB], F32R, tag="x")
                nc.sync.dma_start(out=x_s, in_=x0t[:, it * NB:(it + 4) * NB])
                eg_s = io.tile([128, 4, NSUB, GD], F32, tag="eg")
                nc.sync.dma_start(
                    out=eg_s,
                    in_=eg[:, it * NSUB * GD:(it + 4) * NSUB * GD]
                    .rearrange("p (t s e) -> p t s e", t=4, s=NSUB))
            if g8 == 0:
                rnd_s = io.tile([128, 8, NSUB, D], F32, tag="rnd")
                nc.sync.dma_start(
                    out=rnd_s,
                    in_=rnd[:, it * NSUB * D:(it + 8) * NSUB * D]
                    .rearrange("p (t s d) -> p t s d", t=8, s=NSUB))
                stage = io.tile([128, 8, NSUB, D], F32, tag="stage")

            # --- trunk matmuls + relus ---
            h1p = ptrunk.tile([128, NB], F32, tag="pt")
            nc.tensor.matmul(h1p, lhsT=w1t_s[:, :],
                             rhs=x_s[:, g4 * NB:(g4 + 1) * NB],
                             start=True, stop=True)
            h1 = act.tile([128, NB], F32R, tag="h1")
            nc.scalar.activation(h1, h1p, func=mybir.ActivationFunctionType.Relu,
                                 bias=b1_s[:, :], scale=1.0)

            h2ap = ptrunk.tile([128, NB], F32, tag="pt")
            nc.tensor.matmul(h2ap, lhsT=w2t_s[:, 0:128], rhs=h1[:, :],
                             start=True, stop=True)
            h2a = act.tile([128, NB], F32R, tag="h2a")
            nc.scalar.activation(h2a, h2ap, func=mybir.ActivationFunctionType.Relu,
                                 bias=b2a_s[:, :], scale=1.0)

            h2bp = ptrunk.tile([128, NB], F32, tag="pt")
            nc.tensor.matmul(h2bp, lhsT=w2t_s[:, 128:256], rhs=h1[:, :],
                             start=True, stop=True)
            h2b = act.tile([128, NB], F32R, tag="h2b")
            nc.scalar.activation(h2b, h2bp, func=mybir.ActivationFunctionType.Relu,
                                 bias=b2b_s[:, :], scale=1.0)

            h3ap = ptrunk.tile([128, NB], F32, tag="pt")
            nc.tensor.matmul(h3ap, lhsT=w3ta_s[:, 0:128], rhs=h2a[:, :],
                             start=True, stop=False)
            nc.tensor.matmul(h3ap, lhsT=w3tb_s[:, 0:128], rhs=h2b[:, :],
                             start=False, stop=True)
            h3a = act.tile([128, NB], F32R, tag="h3a")
            nc.scalar.activation(h3a, h3ap, func=mybir.ActivationFunctionType.Relu,
                                 bias=b3a_s[:, :], scale=1.0)

            h3bp = ptrunk.tile([72, NB], F32, tag="pt")
            nc.tensor.matmul(h3bp, lhsT=w3ta_s[:, 128:200], rhs=h2a[:, :],
                             start=True, stop=False)
            nc.tensor.matmul(h3bp, lhsT=w3tb_s[:, 128:200], rhs=h2b[:, :],
                             start=False, stop=True)
            h3b = h3b_t[it % 3]
            nc.scalar.activation(h3b[0:72, :], h3bp,
                                 func=mybir.ActivationFunctionType.Relu,
                                 bias=b3b_s[:, :], scale=1.0)

            # --- heads: two psum tiles of [128, 2sub, 512]; cols 0:300 =
            #     [mu|sig|pai] in (d*25+g) order; bias rides the ones-row ---
            ph = []
            for half in range(2):
                pht = pheads.tile([128, 2, 512], F32, tag="ph")
                for j in range(2):
                    s = half * 2 + j
                    c0, c1 = s * 128, (s + 1) * 128
                    nc.tensor.matmul(pht[:, j, 0:300], lhsT=h3a[:, c0:c1],
                                     rhs=wha_s[:, :], start=True, stop=False)
                    nc.tensor.matmul(pht[:, j, 0:300], lhsT=h3b[:, c0:c1],
                                     rhs=whb_s[:, :], start=False, stop=True)
                ph.append(pht)

            egs = eg_s[:, g4]                       # [128, NSUB, GD]
            # --- evacuate [mu|sig] from PSUM early (frees head banks for the
            #     next tile): fp16 copies split across Act and DVE ---
            valM = samp.tile([128, NSUB, 2 * GD], F16, tag="valM")
            nc.scalar.activation(valM[:, 0:2], ph[0][:, :, 0:200],
                                 func=mybir.ActivationFunctionType.Copy,
                                 bias=0.0, scale=1.0)
            nc.vector.tensor_copy(valM[:, 2:4], ph[1][:, :, 0:200])

            # --- score t = pai_raw * eg  (DVE; one PSUM input each) ---
            t = samp.tile([128, NSUB, GD], F32, tag="t")
            nc.vector.tensor_tensor(out=t[:, 0:2], in0=ph[0][:, :, 200:300],
                                    in1=egs[:, 0:2], op=A.mult)
            nc.vector.tensor_tensor(out=t[:, 2:4], in0=ph[1][:, :, 200:300],
                                    in1=egs[:, 2:4], op=A.mult)

            # --- sc = t*t (Pool; squares rank like |t| since eg > 0) ---
            sc = samp.tile([128, NSUB, GD], F32, tag="sc")
            nc.gpsimd.tensor_tensor(out=sc, in0=t, in1=t, op=A.mult)

            # --- group max over g (DVE reduce on (d g) layout) ---
            smax = samp.tile([128, NSUB, D], F32, tag="smax")
            sc_v = sc.rearrange("p s (d g) -> p s d g", d=D)
            nc.vector.tensor_reduce(smax, sc_v, axis=AX.X, op=A.max)

            # --- diff = sc - smax (Pool, per-sub to keep APs <=3D) ---
            dif = samp.tile([128, NSUB, GD], F32, tag="dif")
            for s in range(NSUB):
                smax_b = smax[:, s].unsqueeze(2).broadcast_to([128, D, G])
                nc.gpsimd.tensor_tensor(
                    out=dif[:, s].rearrange("p (d g) -> p d g", d=D),
                    in0=sc_v[:, s], in1=smax_b, op=A.subtract)
            oh = samp.tile([128, NSUB, GD], F16, tag="oh")
            nc.gpsimd.tensor_scalar(out=oh, in0=dif, scalar1=0.0, scalar2=None,
                                    op0=A.is_ge)

            # --- masked select: m = oh * [mu|sig] (fp16, 2x DVE); zero-sum
            #     reduce-add picks the selected value. m is h-major. ---
            m = samp.tile([128, 2, NSUB, GD], F16, tag="m")
            for h in range(2):
                nc.vector.tensor_tensor(
                    out=m[:, h], in0=oh,
                    in1=valM.rearrange("p s (h e) -> p s h e", h=2)[:, :, h],
                    op=A.mult)
            sel = samp.tile([128, 2, NSUB, D], F32, tag="sel")
            nc.vector.tensor_reduce(
                sel, m.rearrange("p h s (d g) -> p h s d g", d=D),
                axis=AX.X, op=A.add)

            # --- out = rnd*|sig_sel| + mu_sel ---
            sg = samp.tile([128, NSUB, D], F32, tag="sg")
            nc.vector.scalar_tensor_tensor(out=sg, in0=sel[:, 1], scalar=-1.0,
                                           op0=A.mult, op1=A.max, in1=sel[:, 1])
            t1 = samp.tile([128, NSUB, D], F32, tag="t1")
            nc.gpsimd.tensor_tensor(out=t1, in0=sg, in1=rnd_s[:, g8], op=A.mult)
            nc.gpsimd.tensor_tensor(out=stage[:, g8], in0=t1, in1=sel[:, 0],
                                    op=A.add)

            if g8 == 7:
                nc.sync.dma_start(
                    out=outd[:, (it - 7) * NSUB * D:(it + 1) * NSUB * D]
                    .rearrange("p (t s d) -> p t s d", t=8, s=NSUB),
                    in_=stage)

    _split_multi_waits(nc)
    return nc


_NC_CACHE = None
LAST_RESULT = None


def kernel(x0, rand, gumbel, W1, b1, W2, b2, W3, b3,
           Wmu, bmu, Wsig, bsig, Wpai, bpai):
    global _NC_CACHE, LAST_RESULT
    if _NC_CACHE is None:
        _NC_CACHE = _build_nc()
    nc = _NC_CACHE

    x0 = np.ascontiguousarray(np.asarray(x0, np.float32))
    rand = np.ascontiguousarray(np.asarray(rand, np.float32))
    gumbel = np.asarray(gumbel, np.float32)

    # Head weight block [201, 300]: rows 0..199 = h3 feats, row 200 = bias.
    # col = head*100 + d*25 + g  (d-major, g contiguous for segmented ops)
    WH = np.zeros((H3 + 1, 300), np.float32)
    for hd, (W, bvec) in enumerate([(Wmu, bmu), (Wsig, bsig), (Wpai, bpai)]):
        Wt = np.asarray(W, np.float32).transpose(1, 0, 2).reshape(GD, H3)  # (d g) rows
        WH[:H3, hd * GD:(hd + 1) * GD] = Wt.T
        WH[H3, hd * GD:(hd + 1) * GD] = np.asarray(bvec, np.float32).T.reshape(GD)

    wmats = {
        "w1t": np.ascontiguousarray(np.asarray(W1, np.float32).T),
        "b1": np.asarray(b1, np.float32).reshape(H1, 1),
        "w2t": np.ascontiguousarray(np.asarray(W2, np.float32).T),
        "b2": np.asarray(b2, np.float32).reshape(H2, 1),
        "w3t": np.ascontiguousarray(np.asarray(W3, np.float32).T),
        "b3": np.asarray(b3, np.float32).reshape(H3, 1),
        "wha": np.ascontiguousarray(WH[0:128]),
        "onesr": np.ones((1, NB), np.float32),
        "whb": np.ascontiguousarray(WH[128:201]),
    }

    eg_full = np.exp(gumbel, dtype=np.float32)      # [B, G, D]

    in_maps = []
    for c in range(NCORES):
        sl = slice(c * BS, (c + 1) * BS)
        # eg: [BS,G,D] -> (d g) cols -> [p, it, s, e] contiguous per partition
        egc = eg_full[sl].transpose(0, 2, 1).reshape(BS, GD)
        egc = egc.reshape(NT, NSUB, 128, GD).transpose(2, 0, 1, 3)
        rndc = rand[sl].reshape(NT, NSUB, 128, D).transpose(2, 0, 1, 3)
        m = {
            "x0t": np.ascontiguousarray(x0[sl].T),
            "eg": np.ascontiguousarray(egc.reshape(128, NT * NSUB * GD)),
            "rnd": np.ascontiguousarray(rndc.reshape(128, NT * NSUB * D)),
        }
        m.update(wmats)
        in_maps.append(m)

    res = run_bass_kernel_spmd(nc, in_maps, core_ids=list(range(NCORES)))
    LAST_RESULT = res
    outs = []
    for c in range(NCORES):
        oc = res.results[c]["outd"].reshape(128, NT, NSUB, D)
        outs.append(oc.transpose(1, 2, 0, 3).reshape(BS, D))
    return np.concatenate(outs, axis=0).astype(np.float32)


# revision 15
# speedup vs baseline: 1.2863x; 1.0106x over previous
"""Trainium2 Bass kernel for nn_Backward_12094627905824 (MLP trunk + gumbel-argmax
mixture sampling). Data-parallel over 8 NeuronCores: batch B=262144 sharded
32768 rows/core; small MLP / head weights replicated.

Per batch row b:
  h = relu chain 3 -> 128 -> 256 -> 200
  mu/sig/pai[g,d] = heads (25 comps x 4 dims)
  idx[d] = argmax_g log(|pai|+eps) + gumbel[b,g,d]
  out[b,d] = rand[b,d]*|sig[idx,d]| + mu[idx,d]

Device reformulation (argmax-invariant): score t = pai_raw * exp(gumbel)
(exp precomputed on host), sc = |t|; one-hot oh = (sc >= groupmax); select
mu/sig via additive mask m = oh*1024 + val, reduce-max, subtract 1024.

Engine split per 512-row tile (balanced against the TimelineSim cost model):
  PE   : 15 matmuls (trunk + heads; head biases ride a ones-row in h3b)
  Act  : 5 relu+bias PSUM->SBUF copies
  DVE  : score mult (PSUM), group max, masked select stt + reduce-max
  Pool : |t| via abs_max, diff vs groupmax, one-hot u8, output combine
All head columns are (d*25+g)-ordered; eg is host-transposed to a per-partition
contiguous layout so DMAs move >=512B elements (no descriptor penalty) and are
batched 4-8 tiles per DMA instruction.
"""
import numpy as np

import concourse.bass as bass
import concourse.mybir as mybir
import bass_rust
from concourse.tile import TileContext
from concourse.bass_utils import run_bass_kernel_spmd

NCORES = 8
B, G, D = 262144, 25, 4
GD = G * D                       # 100
H1, H2, H3 = 128, 256, 200
BS = B // NCORES                 # 32768 rows per core
NB = 512                         # batch columns per compute tile
NT = BS // NB                    # 64 tiles
NSUB = NB // 128                 # 4 sub-blocks of 128 rows
BIG = 1024.0                     # additive mask offset (>> |mu|,|sig|)

F32 = mybir.dt.float32
F32R = mybir.dt.float32r
U8 = mybir.dt.uint8
F16 = mybir.dt.float16
A = mybir.AluOpType
AX = mybir.AxisListType


def _split_multi_waits(nc):
    # walrus CoreV3 codegen accepts only one sync-wait per instruction; Tile's
    # exit drain waits once per active proc. Split into single-wait drains.
    for bb in nc.main_func.blocks:
        insts = list(bb.instructions)
        out = []
        changed = False
        for inst in insts:
            si = inst.sync_info
            if si is not None and len(si.on_wait) > 1:
                waits = list(si.on_wait)
                for k, w in enumerate(waits[:-1]):
                    d = mybir.InstDrain(name=f"{inst.name}-sw{k}", ins=[], outs=[])
                    d.engine = inst.engine
                    d.sync_info = bass_rust.SyncInfo(on_wait=[w], on_update=[])
                    nc.register_instruction(d)
                    out.append(d)
                si.on_wait = [waits[-1]]
                changed = True
            out.append(inst)
        if changed:
            bb.instructions = out


def _build_nc():
    nc = bass.Bass(trn_type="TRN2")

    x0t = nc.dram_tensor("x0t", [3, BS], F32R, kind="ExternalInput")
    eg = nc.dram_tensor("eg", [128, NT * NSUB * GD], F32, kind="ExternalInput")
    rnd = nc.dram_tensor("rnd", [128, NT * NSUB * D], F32, kind="ExternalInput")
    w1t = nc.dram_tensor("w1t", [3, H1], F32R, kind="ExternalInput")
    b1 = nc.dram_tensor("b1", [H1, 1], F32, kind="ExternalInput")
    w2t = nc.dram_tensor("w2t", [H1, H2], F32R, kind="ExternalInput")
    b2 = nc.dram_tensor("b2", [H2, 1], F32, kind="ExternalInput")
    w3t = nc.dram_tensor("w3t", [H2, H3], F32R, kind="ExternalInput")
    b3 = nc.dram_tensor("b3", [H3, 1], F32, kind="ExternalInput")
    wha = nc.dram_tensor("wha", [128, 300], F32R, kind="ExternalInput")
    whb = nc.dram_tensor("whb", [73, 300], F32R, kind="ExternalInput")
    onesr = nc.dram_tensor("onesr", [1, NB], F32R, kind="ExternalInput")
    segmask = nc.dram_tensor("segmask", [128, 2 * NSUB * GD], mybir.dt.float16,
                             kind="ExternalInput")
    outd = nc.dram_tensor("outd", [128, NT * NSUB * D], F32, kind="ExternalOutput")

    from contextlib import ExitStack
    with TileContext(nc) as tc, ExitStack() as ctx:
        const = ctx.enter_context(tc.tile_pool(name="const", bufs=1))
        io = ctx.enter_context(tc.tile_pool(name="io", bufs=3))
        act = ctx.enter_context(tc.tile_pool(name="act", bufs=4))
        samp = ctx.enter_context(tc.tile_pool(name="samp", bufs=4))
        ptrunk = ctx.enter_context(tc.tile_pool(name="ptrunk", bufs=2, space="PSUM"))
        pheads = ctx.enter_context(tc.tile_pool(name="pheads", bufs=3, space="PSUM"))

        # --- load weights once ---
        w1t_s = const.tile([3, H1], F32R)
        nc.sync.dma_start(out=w1t_s, in_=w1t[:, :])
        b1_s = const.tile([H1, 1], F32)
        nc.sync.dma_start(out=b1_s, in_=b1[:, :])
        w2t_s = const.tile([H1, H2], F32R)
        nc.sync.dma_start(out=w2t_s, in_=w2t[:, :])
        b2a_s = const.tile([128, 1], F32, tag="b2a")
        nc.sync.dma_start(out=b2a_s, in_=b2[0:128, :])
        b2b_s = const.tile([128, 1], F32, tag="b2b")
        nc.sync.dma_start(out=b2b_s, in_=b2[128:256, :])
        w3ta_s = const.tile([128, H3], F32R, tag="w3ta")   # h2 feats 0:128
        nc.sync.dma_start(out=w3ta_s, in_=w3t[0:128, :])
        w3tb_s = const.tile([128, H3], F32R, tag="w3tb")   # h2 feats 128:256
        nc.sync.dma_start(out=w3tb_s, in_=w3t[128:256, :])
        b3a_s = const.tile([128, 1], F32, tag="b3a")
        nc.sync.dma_start(out=b3a_s, in_=b3[0:128, :])
        b3b_s = const.tile([72, 1], F32, tag="b3b")
        nc.sync.dma_start(out=b3b_s, in_=b3[128:200, :])
        wha_s = const.tile([128, 300], F32R, tag="wha")    # h3 feats 0:128
        nc.sync.dma_start(out=wha_s, in_=wha[:, :])
        whb_s = const.tile([73, 300], F32R, tag="whb")     # h3 feats 128:200 + bias row
        nc.sync.dma_start(out=whb_s, in_=whb[:, :])

        segm_s = const.tile([128, 2 * NSUB * GD], F16, tag="segm")
        nc.sync.dma_start(out=segm_s, in_=segmask[:, :])

        # h3b tiles with a constant ones-row at partition 72 (manual 2-buffer)
        h3b_t = []
        for k in range(3):
            t = const.tile([73, NB], F32R, tag=f"h3b{k}")
            nc.sync.dma_start(out=t[72:73, :], in_=onesr[:, :])
            h3b_t.append(t)

        for it in range(NT):
            g4 = it % 4      # position in 4-tile eg/x DMA group
            g8 = it % 8      # position in 8-tile rnd/out DMA group

            # --- batched input DMAs ---
            if g4 == 0:
                x_s = io.tile([3, 4 * NB], F32R, tag="x")
                nc.sync.dma_start(out=x_s, in_=x0t[:, it * NB:(it + 4) * NB])
                eg_s = io.tile([128, 4, NSUB, GD], F32, tag="eg")
                nc.sync.dma_start(
                    out=eg_s,
                    in_=eg[:, it * NSUB * GD:(it + 4) * NSUB * GD]
                    .rearrange("p (t s e) -> p t s e", t=4, s=NSUB))
            if g8 == 0:
                rnd_s = io.tile([128, 8, NSUB, D], F32, tag="rnd")
                nc.sync.dma_start(
                    out=rnd_s,
                    in_=rnd[:, it * NSUB * D:(it + 8) * NSUB * D]
                    .rearrange("p (t s d) -> p t s d", t=8, s=NSUB))
                stage = io.tile([128, 8, NSUB, D], F32, tag="stage")

            # --- trunk matmuls + relus ---
            h1p = ptrunk.tile([128, NB], F32, tag="pt")
            nc.tensor.matmul(h1p, lhsT=w1t_s[:, :],
                             rhs=x_s[:, g4 * NB:(g4 + 1) * NB],
                             start=True, stop=True)
            h1 = act.tile([128, NB], F32R, tag="h1")
            nc.scalar.activation(h1, h1p, func=mybir.ActivationFunctionType.Relu,
                                 bias=b1_s[:, :], scale=1.0)

            h2ap = ptrunk.tile([128, NB], F32, tag="pt")
            nc.tensor.matmul(h2ap, lhsT=w2t_s[:, 0:128], rhs=h1[:, :],
                             start=True, stop=True)
            h2a = act.tile([128, NB], F32R, tag="h2a")
            nc.scalar.activation(h2a, h2ap, func=mybir.ActivationFunctionType.Relu,
                                 bias=b2a_s[:, :], scale=1.0)

            h2bp = ptrunk.tile([128, NB], F32, tag="pt")
            nc.tensor.matmul(h2bp, lhsT=w2t_s[:, 128:256], rhs=h1[:, :],
                             start=True, stop=True)
            h2b = act.tile([128, NB], F32R, tag="h2b")
            nc.scalar.activation(h2b, h2bp, func=mybir.ActivationFunctionType.Relu,
                                 bias=b2b_s[:, :], scale=1.0)

            h3ap = ptrunk.tile([128, NB], F32, tag="pt")
            nc.tensor.matmul(h3ap, lhsT=w3ta_s[:, 0:128], rhs=h2a[:, :],
                             start=True, stop=False)
            nc.tensor.matmul(h3ap, lhsT=w3tb_s[:, 0:128], rhs=h2b[:, :],
                             start=False, stop=True)
            h3a = act.tile([128, NB], F32R, tag="h3a")
            nc.scalar.activation(h3a, h3ap, func=mybir.ActivationFunctionType.Relu,
                                 bias=b3a_s[:, :], scale=1.0)

            h3bp = ptrunk.tile([72, NB], F32, tag="pt")
            nc.tensor.matmul(h3bp, lhsT=w3ta_s[:, 128:200], rhs=h2a[:, :],
                             start=True, stop=False)
            nc.tensor.matmul(h3bp, lhsT=w3tb_s[:, 128:200], rhs=h2b[:, :],
                             start=False, stop=True)
            h3b = h3b_t[it % 3]
            nc.scalar.activation(h3b[0:72, :], h3bp,
                                 func=mybir.ActivationFunctionType.Relu,
                                 bias=b3b_s[:, :], scale=1.0)

            # --- heads: two psum tiles of [128, 2sub, 512]; cols 0:300 =
            #     [mu|sig|pai] in (d*25+g) order; bias rides the ones-row ---
            ph = []
            for half in range(2):
                pht = pheads.tile([128, 2, 512], F32, tag="ph")
                for j in range(2):
                    s = half * 2 + j
                    c0, c1 = s * 128, (s + 1) * 128
                    nc.tensor.matmul(pht[:, j, 0:300], lhsT=h3a[:, c0:c1],
                                     rhs=wha_s[:, :], start=True, stop=False)
                    nc.tensor.matmul(pht[:, j, 0:300], lhsT=h3b[:, c0:c1],
                                     rhs=whb_s[:, :], start=False, stop=True)
                ph.append(pht)

            egs = eg_s[:, g4]                       # [128, NSUB, GD]
            # --- evacuate [mu|sig] from PSUM early (frees head banks for the
            #     next tile): fp16 copies split across Act and DVE ---
            valM = samp.tile([128, NSUB, 2 * GD], F16, tag="valM")
            nc.vector.tensor_copy(valM[:, 0:2], ph[0][:, :, 0:200])
            nc.vector.tensor_copy(valM[:, 2:4], ph[1][:, :, 0:200])

            # --- score t = pai_raw * eg  (DVE; one PSUM input each) ---
            t = samp.tile([128, NSUB, GD], F32, tag="t")
            nc.vector.tensor_tensor(out=t[:, 0:2], in0=ph[0][:, :, 200:300],
                                    in1=egs[:, 0:2], op=A.mult)
            nc.vector.tensor_tensor(out=t[:, 2:4], in0=ph[1][:, :, 200:300],
                                    in1=egs[:, 2:4], op=A.mult)

            # --- sc = t*t (Pool; squares rank like |t| since eg > 0) ---
            sc = samp.tile([128, NSUB, GD], F32, tag="sc")
            nc.gpsimd.tensor_tensor(out=sc, in0=t, in1=t, op=A.mult)

            # --- group max over g (DVE reduce on (d g) layout) ---
            smax = samp.tile([128, NSUB, D], F32, tag="smax")
            sc_v = sc.rearrange("p s (d g) -> p s d g", d=D)
            nc.vector.tensor_reduce(smax, sc_v, axis=AX.X, op=A.max)

            # --- diff = sc - smax (Pool, per-sub to keep APs <=3D) ---
            dif = samp.tile([128, NSUB, GD], F32, tag="dif")
            for s in range(NSUB):
                smax_b = smax[:, s].unsqueeze(2).broadcast_to([128, D, G])
                nc.gpsimd.tensor_tensor(
                    out=dif[:, s].rearrange("p (d g) -> p d g", d=D),
                    in0=sc_v[:, s], in1=smax_b, op=A.subtract)
            oh = samp.tile([128, NSUB, GD], F16, tag="oh")
            nc.gpsimd.tensor_scalar(out=oh, in0=dif, scalar1=0.0, scalar2=None,
                                    op0=A.is_ge)

            # --- masked select: m = oh * [mu|sig] (fp16, 2x DVE); zero-sum
            #     reduce-add picks the selected value. m is h-major. ---
            m = samp.tile([128, 2, NSUB, GD], F16, tag="m")
            for h in range(2):
                nc.vector.tensor_tensor(
                    out=m[:, h], in0=oh,
                    in1=valM.rearrange("p s (h e) -> p s h e", h=2)[:, :, h],
                    op=A.mult)
            msc = samp.tile([128, 2, NSUB, GD], F16, tag="msc")
            nc.vector.tensor_tensor_scan(
                out=msc.rearrange("p h s e -> p (h s e)"),
                data0=segm_s, data1=m.rearrange("p h s e -> p (h s e)"),
                initial=0.0, op0=A.mult, op1=A.add)
            msc_v = msc.rearrange("p h s (d g) -> p h s d g", d=D)
            sel_mu = msc_v[:, 0, :, :, G - 1]          # [128, NSUB, D] strided
            sel_sig = msc_v[:, 1, :, :, G - 1]

            # --- out = rnd*|sig_sel| + mu_sel ---
            sg = samp.tile([128, NSUB, D], F32, tag="sg")
            nc.vector.scalar_tensor_tensor(out=sg, in0=sel_sig, scalar=-1.0,
                                           op0=A.mult, op1=A.max, in1=sel_sig)
            t1 = samp.tile([128, NSUB, D], F32, tag="t1")
            nc.gpsimd.tensor_tensor(out=t1, in0=sg, in1=rnd_s[:, g8], op=A.mult)
            nc.gpsimd.tensor_tensor(out=stage[:, g8], in0=t1, in1=sel_mu,
                                    op=A.add)

            if g8 == 7:
                nc.sync.dma_start(
                    out=outd[:, (it - 7) * NSUB * D:(it + 1) * NSUB * D]
                    .rearrange("p (t s d) -> p t s d", t=8, s=NSUB),
                    in_=stage)

    _split_multi_waits(nc)
    return nc


_NC_CACHE = None
LAST_RESULT = None


def kernel(x0, rand, gumbel, W1, b1, W2, b2, W3, b3,
           Wmu, bmu, Wsig, bsig, Wpai, bpai):
    global _NC_CACHE, LAST_RESULT
    if _NC_CACHE is None:
        _NC_CACHE = _build_nc()
    nc = _NC_CACHE

    x0 = np.ascontiguousarray(np.asarray(x0, np.float32))
    rand = np.ascontiguousarray(np.asarray(rand, np.float32))
    gumbel = np.asarray(gumbel, np.float32)

    # Head weight block [201, 300]: rows 0..199 = h3 feats, row 200 = bias.
    # col = head*100 + d*25 + g  (d-major, g contiguous for segmented ops)
    WH = np.zeros((H3 + 1, 300), np.float32)
    for hd, (W, bvec) in enumerate([(Wmu, bmu), (Wsig, bsig), (Wpai, bpai)]):
        Wt = np.asarray(W, np.float32).transpose(1, 0, 2).reshape(GD, H3)  # (d g) rows
        WH[:H3, hd * GD:(hd + 1) * GD] = Wt.T
        WH[H3, hd * GD:(hd + 1) * GD] = np.asarray(bvec, np.float32).T.reshape(GD)

    wmats = {
        "w1t": np.ascontiguousarray(np.asarray(W1, np.float32).T),
        "b1": np.asarray(b1, np.float32).reshape(H1, 1),
        "w2t": np.ascontiguousarray(np.asarray(W2, np.float32).T),
        "b2": np.asarray(b2, np.float32).reshape(H2, 1),
        "w3t": np.ascontiguousarray(np.asarray(W3, np.float32).T),
        "b3": np.asarray(b3, np.float32).reshape(H3, 1),
        "wha": np.ascontiguousarray(WH[0:128]),
        "onesr": np.ones((1, NB), np.float32),
        "segmask": np.broadcast_to(
            (np.arange(2 * NSUB * GD) % G != 0).astype(np.float16),
            (128, 2 * NSUB * GD)).copy(),
        "whb": np.ascontiguousarray(WH[128:201]),
    }

    eg_full = np.exp(gumbel, dtype=np.float32)      # [B, G, D]

    in_maps = []
    for c in range(NCORES):
        sl = slice(c * BS, (c + 1) * BS)
        # eg: [BS,G,D] -> (d g) cols -> [p, it, s, e] contiguous per partition
        egc = eg_full[sl].transpose(0, 2, 1).reshape(BS, GD)
        egc = egc.reshape(NT, NSUB, 128, GD).transpose(2, 0, 1, 3)
        rndc = rand[sl].reshape(NT, NSUB, 128, D).transpose(2, 0, 1, 3)
        m = {
            "x0t": np.ascontiguousarray(x0[sl].T),
            "eg": np.ascontiguousarray(egc.reshape(128, NT * NSUB * GD)),
            "rnd": np.ascontiguousarray(rndc.reshape(128, NT * NSUB * D)),
        }
        m.update(wmats)
        in_maps.append(m)

    res = run_bass_kernel_spmd(nc, in_maps, core_ids=list(range(NCORES)))
    LAST_RESULT = res
    outs = []
    for c in range(NCORES):
        oc = res.results[c]["outd"].reshape(128, NT, NSUB, D)
        outs.append(oc.transpose(1, 2, 0, 3).reshape(BS, D))
    return np.concatenate(outs, axis=0).astype(np.float32)
